# revision 15
# baseline (speedup 1.0000x reference)
"""Trainium2 Bass kernel for nn_Decoder (gnn_message_passing).

Strategy (per core, data-parallel over batch B=128 across 8 cores, 16 samples each):
  - Activations flow "transposed" (features on partitions) so every weight
    matrix is used in its natural [din, dout] layout as the matmul lhsT and
    biases are per-partition ACT bias operands.
  - The edge MLP's pair-gather (enc[ii]+enc[jj]) is computed as a matmul
    against a constant 0/1 incidence matrix with an extra all-ones row that
    folds in the per-sample latent contribution + bias (K=65).
  - Matmul operands are float32r (1 cycle/row for N>=256, ~1e-4 rel err).
  - The symmetric adjacency is assembled from the upper-tri values with
    row-block DMAs + PE transposes.
"""

import sys

sys.path.insert(0, "/opt/trn_rl_repo")

import numpy as np

import concourse.bass as bass
import concourse.tile as tile
from concourse import bacc, mybir
from concourse.bass_utils import run_bass_kernel_spmd
from concourse.masks import make_identity

F32 = mybir.dt.float32
MM = mybir.dt.float32r     # matmul operand dtype
LR = mybir.ActivationFunctionType.Prelu
IDN = mybir.ActivationFunctionType.Identity

N_CORES = 8
B = 128
BL = B // N_CORES          # 16 samples per core
NN = 64                    # nodes
NF = 16                    # node features
E = NN * (NN - 1) // 2     # 2016 edges
LAT = 128

_II, _JJ = np.triu_indices(NN, k=1)
_BASE = np.concatenate([[0], np.cumsum(63 - np.arange(NN))]).astype(int)


def _build_program(er_b3_val: float):
    nc = bacc.Bacc(None)

    p = {}
    # (name, shape, dtype) — MM-typed params feed matmuls, F32 ones feed ACT bias
    ins_specs = [
        ("dataT", [LAT, BL], MM),
        ("nr_w1", [128, 512], MM), ("nr_w2", [512, 1024], MM), ("nr_w3", [1024, 1088], MM),
        ("nr_b1", [128, 4], F32), ("nr_b2", [128, 8], F32), ("nr_b3", [1, 1088], MM),
        ("ne_w1", [16, 128], MM), ("ne_w2", [128, 128], MM),
        ("ne_b1", [128, 1], F32), ("ne_b2", [128, 1], F32),
        ("pl_w1", [128, 256], MM), ("pl_w2", [256, 128], MM),
        ("pl_b1", [128, 2], F32), ("pl_b2", [128, 1], F32),
        ("w1a", [128, 256], MM), ("w1b", [128, 256], MM), ("er_b1", [1, 256], MM),
        ("er_w2", [256, 256], MM), ("er_b2", [128, 2], F32),
        ("er_w3", [256, 1], MM),
        ("incm", [NN + 1, E], MM),
        ("ones16", [1, BL], MM),
    ]
    for nm, shape, dt in ins_specs:
        p[nm] = nc.declare_dram_parameter(nm, shape, dt, isOutput=False)
    out_nodes = nc.declare_dram_parameter("nodes", [BL, 1088], F32, isOutput=True)
    out_adj = nc.declare_dram_parameter("adj", [BL, NN * NN], F32, isOutput=True)
    c_bounce = nc.dram_tensor("c_bounce", [1, BL, 256], MM)
    vals_d = nc.dram_tensor("vals_d", [1, BL, E], F32)

    with tile.TileContext(nc) as tc:
        with (
            tc.tile_pool(name="w", bufs=1) as wp,
            tc.tile_pool(name="acts", bufs=1) as ap_,
        ):
            # ---- persistent small weights / constants ----
            t = {}
            for nm, shape, dt in [("ne_w1", [16, 128], MM), ("ne_w2", [128, 128], MM),
                                  ("ne_b1", [128, 1], F32), ("ne_b2", [128, 1], F32),
                                  ("pl_w1", [128, 256], MM),
                                  ("pl_b1", [128, 2], F32), ("pl_b2", [128, 1], F32),
                                  ("w1a", [128, 256], MM), ("w1b", [128, 256], MM),
                                  ("er_b1", [1, 256], MM), ("er_b2", [128, 2], F32),
                                  ("incm", [NN + 1, E], MM), ("dataT", [LAT, BL], MM)]:
                t[nm] = wp.tile(shape, dt, tag=nm, name=nm)
                nc.sync.dma_start(out=t[nm][:], in_=p[nm][:])
            t["pl_w2"] = [wp.tile([128, 128], MM, tag=f"pl_w2_{k}", name=f"pl_w2_{k}")
                          for k in range(2)]
            t["er_w2"] = [wp.tile([128, 256], MM, tag=f"er_w2_{k}", name=f"er_w2_{k}")
                          for k in range(2)]
            t["er_w3"] = [wp.tile([128, 1], MM, tag=f"er_w3_{k}", name=f"er_w3_{k}")
                          for k in range(2)]
            for k in range(2):
                nc.sync.dma_start(out=t["pl_w2"][k][:], in_=p["pl_w2"][k * 128:(k + 1) * 128, :])
                nc.sync.dma_start(out=t["er_w2"][k][:], in_=p["er_w2"][k * 128:(k + 1) * 128, :])
                nc.sync.dma_start(out=t["er_w3"][k][:], in_=p["er_w3"][k * 128:(k + 1) * 128, :])

            ident = wp.tile([NN, NN], F32, tag="ident", name="ident")
            make_identity(nc, ident[:])
            ones16 = wp.tile([1, BL], MM, tag="ones16", name="ones16")
            nc.sync.dma_start(out=ones16[:], in_=p["ones16"][:])

            # persistent activations (outlive phase B)
            t_QB = ap_.tile([NN + 1, BL * 256], MM, tag="QB", name="t_QB")
            t_adjU = ap_.tile([NN, BL, NN], F32, tag="adjU", name="t_adjU")
            nc.gpsimd.memset(t_adjU[:], 0.0)

            # ================= phase B: small MLPs =================
            with (
                tc.tile_pool(name="bigw", bufs=1) as bw,
                tc.tile_pool(name="psB", bufs=3, space="PSUM") as pB,
            ):
                t_nrw1 = bw.tile([128, 512], MM, tag="nr_w1", name="t_nrw1")
                nc.sync.dma_start(out=t_nrw1[:], in_=p["nr_w1"][:])
                t_nrw2 = [bw.tile([128, 1024], MM, tag=f"nr_w2_{k}", name=f"t_nrw2_{k}")
                          for k in range(4)]
                for k in range(4):
                    nc.sync.dma_start(out=t_nrw2[k][:], in_=p["nr_w2"][k * 128:(k + 1) * 128, :])
                t_nrw3 = [bw.tile([128, 1088], MM, tag=f"nr_w3_{k}", name=f"t_nrw3_{k}")
                          for k in range(8)]
                for k in range(8):
                    nc.sync.dma_start(out=t_nrw3[k][:], in_=p["nr_w3"][k * 128:(k + 1) * 128, :])
                t_b = {}
                for nm, shape, dt in [("nr_b1", [128, 4], F32), ("nr_b2", [128, 8], F32),
                                      ("nr_b3", [1, 1088], MM)]:
                    t_b[nm] = bw.tile(shape, dt, tag=nm, name=nm)
                    nc.sync.dma_start(out=t_b[nm][:], in_=p[nm][:])

                # node reconstruction MLP (transposed)
                h1 = [bw.tile([128, BL], MM, tag=f"h1_{m}", name=f"h1_{m}") for m in range(4)]
                for m in range(4):
                    ps = pB.tile([128, 1024], F32, tag="ps", name="psb")
                    nc.tensor.matmul(ps[:, 0:BL], t_nrw1[:, m * 128:(m + 1) * 128],
                                     t["dataT"][:], start=True, stop=True)
                    nc.scalar.activation(h1[m][:], ps[:, 0:BL], LR,
                                         bias=t_b["nr_b1"][:, m:m + 1], alpha=0.01)
                h2 = [bw.tile([128, BL], MM, tag=f"h2_{m}", name=f"h2_{m}") for m in range(8)]
                for m in range(8):
                    ps = pB.tile([128, 1024], F32, tag="ps", name="psb")
                    for k in range(4):
                        nc.tensor.matmul(ps[:, 0:BL], t_nrw2[k][:, m * 128:(m + 1) * 128],
                                         h1[k][:], start=(k == 0), stop=(k == 3))
                    nc.scalar.activation(h2[m][:], ps[:, 0:BL], LR,
                                         bias=t_b["nr_b2"][:, m:m + 1], alpha=0.01)
                # nodes output (normal orientation), bias via ones-row matmul
                t_nodes = bw.tile([BL, 1088], F32, tag="nodes", name="t_nodes")
                for n0, n1 in [(0, 512), (512, 1024), (1024, 1088)]:
                    ps = pB.tile([128, 1024], F32, tag="ps", name="psb")
                    w = n1 - n0
                    for k in range(8):
                        nc.tensor.matmul(ps[0:BL, 0:w], h2[k][:],
                                         t_nrw3[k][:, n0:n1], start=(k == 0), stop=False)
                    nc.tensor.matmul(ps[0:BL, 0:w], ones16[0:1, :],
                                     t_b["nr_b3"][0:1, n0:n1], start=False, stop=True)
                    nc.scalar.activation(t_nodes[:, n0:n1], ps[0:BL, 0:w], IDN)
                nc.sync.dma_start(out=out_nodes[:], in_=t_nodes[:])

                # node-feature transpose [s, n, c] -> [c, s, n] via DRAM round-trip
                t_n16raw = bw.tile([16, BL * NN], F32, tag="n16raw", name="t_n16raw")
                nc.sync.dma_start(
                    out=t_n16raw[:].rearrange("c (s n) -> c s n", s=BL),
                    in_=out_nodes[:, :].rearrange("s (n c) -> c s n", n=NN)[0:NF, :, :])
                t_n16T = bw.tile([16, BL * NN], MM, tag="n16T", name="t_n16T")
                nc.scalar.copy(t_n16T[:], t_n16raw[:])

                # node encoder (samples batched along free dim)
                t_eT = bw.tile([128, BL * NN], MM, tag="eT", name="t_eT")
                for h in range(2):
                    ps = pB.tile([128, 1024], F32, tag="ps", name="psb")
                    sl = slice(h * 512, (h + 1) * 512)
                    nc.tensor.matmul(ps[:, 0:512], t["ne_w1"][:], t_n16T[:, sl],
                                     start=True, stop=True)
                    nc.scalar.activation(t_eT[:, sl], ps[:, 0:512], LR,
                                         bias=t["ne_b1"][:, 0:1], alpha=0.01)
                t_encT = bw.tile([128, BL * NN], MM, tag="encT", name="t_encT")
                for h in range(2):
                    ps = pB.tile([128, 1024], F32, tag="ps", name="psb")
                    sl = slice(h * 512, (h + 1) * 512)
                    nc.tensor.matmul(ps[:, 0:512], t["ne_w2"][:], t_eT[:, sl],
                                     start=True, stop=True)
                    nc.scalar.activation(t_encT[:, sl], ps[:, 0:512], IDN,
                                         bias=t["ne_b2"][:, 0:1])

                # edge prelayer (transposed)
                pT = [bw.tile([128, BL], MM, tag=f"pT_{m}", name=f"pT_{m}") for m in range(2)]
                for m in range(2):
                    ps = pB.tile([128, 1024], F32, tag="ps", name="psb")
                    nc.tensor.matmul(ps[:, 0:BL], t["pl_w1"][:, m * 128:(m + 1) * 128],
                                     t["dataT"][:], start=True, stop=True)
                    nc.scalar.activation(pT[m][:], ps[:, 0:BL], LR,
                                         bias=t["pl_b1"][:, m:m + 1], alpha=0.01)
                encLatT = bw.tile([128, BL], MM, tag="encLatT", name="encLatT")
                ps = pB.tile([128, 1024], F32, tag="ps", name="psb")
                for k in range(2):
                    nc.tensor.matmul(ps[:, 0:BL], t["pl_w2"][k][:], pT[k][:],
                                     start=(k == 0), stop=(k == 1))
                nc.scalar.activation(encLatT[:], ps[:, 0:BL], LR,
                                     bias=t["pl_b2"][:, 0:1], alpha=0.01)

                # c = enc_latent @ W1b + er_b1 (normal orientation)
                t_c = bw.tile([BL, 256], MM, tag="c", name="t_c")
                ps = pB.tile([128, 1024], F32, tag="ps", name="psb")
                nc.tensor.matmul(ps[0:BL, 0:256], encLatT[:], t["w1b"][:],
                                 start=True, stop=False)
                nc.tensor.matmul(ps[0:BL, 0:256], ones16[0:1, :], t["er_b1"][0:1, :],
                                 start=False, stop=True)
                nc.scalar.activation(t_c[:], ps[0:BL, 0:256], IDN)

                # QB[s] = [enc_nodes_s @ 0.5*W1a ; c_s]  (65 x 256 per sample)
                for s2 in range(BL // 2):
                    ps = pB.tile([128, 1024], F32, tag="ps", name="psb")
                    for q in range(2):
                        s = 2 * s2 + q
                        nc.tensor.matmul(ps[0:NN, q * 256:(q + 1) * 256],
                                         t_encT[:, s * NN:(s + 1) * NN], t["w1a"][:],
                                         start=True, stop=True)
                    nc.scalar.activation(t_QB[0:NN, s2 * 512:(s2 + 1) * 512],
                                         ps[0:NN, 0:512], IDN)
                nc.sync.dma_start(out=c_bounce[0, :, :], in_=t_c[:])
                nc.sync.dma_start(out=t_QB[NN:NN + 1, :].rearrange("p (s f) -> p s f", s=BL),
                                  in_=c_bounce[:, :, :])

            # ================= phase C: edge MLP per sample =================
            with (
                tc.tile_pool(name="zpool", bufs=2) as zp,
                tc.tile_pool(name="pU1", bufs=2, space="PSUM") as pU1,
                tc.tile_pool(name="pU2", bufs=1, space="PSUM") as pU2,
                tc.tile_pool(name="pV", bufs=1, space="PSUM") as pV,
            ):
                for s in range(BL):
                    z1 = [zp.tile([128, E], MM, tag=f"z1_{fc}", name=f"z1_{fc}")
                          for fc in range(2)]
                    for fc in range(2):
                        lw = t_QB[0:NN + 1, s * 256 + fc * 128: s * 256 + (fc + 1) * 128]
                        for blk in range(2):
                            ps = pU1.tile([128, 1024], F32, tag="u1", name="psu1")
                            for hf in range(2):
                                e0 = (blk * 2 + hf) * 504
                                nc.tensor.matmul(ps[:, hf * 512:hf * 512 + 504], lw,
                                                 t["incm"][:, e0:e0 + 504],
                                                 start=True, stop=True)
                            src = ps[:].rearrange("p (a b) -> p a b", a=2)[:, :, 0:504]
                            dst = z1[fc][:, blk * 1008:(blk + 1) * 1008].rearrange(
                                "p (a b) -> p a b", a=2)
                            nc.scalar.activation(dst, src, LR, alpha=0.01)
                    z2 = [zp.tile([128, E], MM, tag=f"z2_{mc}", name=f"z2_{mc}")
                          for mc in range(2)]
                    for mc in range(2):
                        for blk in range(2):
                            ps = pU2.tile([128, 1024], F32, tag="u2", name="psu2")
                            for hf in range(2):
                                e0 = (blk * 2 + hf) * 504
                                for kc in range(2):
                                    nc.tensor.matmul(
                                        ps[:, hf * 512:hf * 512 + 504],
                                        t["er_w2"][kc][:, mc * 128:(mc + 1) * 128],
                                        z1[kc][:, e0:e0 + 504],
                                        start=(kc == 0), stop=(kc == 1))
                            src = ps[:].rearrange("p (a b) -> p a b", a=2)[:, :, 0:504]
                            dst = z2[mc][:, blk * 1008:(blk + 1) * 1008].rearrange(
                                "p (a b) -> p a b", a=2)
                            nc.scalar.activation(dst, src, LR,
                                                 bias=t["er_b2"][:, mc:mc + 1], alpha=0.01)
                    t_vs = zp.tile([1, E], F32, tag="vs", name="t_vs")
                    for blk in range(2):
                        ps = pV.tile([1, 1024], F32, tag="v", name="psv")
                        for hf in range(2):
                            e0 = (blk * 2 + hf) * 504
                            for kc in range(2):
                                nc.tensor.matmul(ps[0:1, hf * 512:hf * 512 + 504],
                                                 t["er_w3"][kc][:],
                                                 z2[kc][:, e0:e0 + 504],
                                                 start=(kc == 0), stop=(kc == 1))
                        src = ps[:].rearrange("p (a b) -> p a b", a=2)[:, :, 0:504]
                        dst = t_vs[0:1, blk * 1008:(blk + 1) * 1008].rearrange(
                            "p (a b) -> p a b", a=2)
                        nc.vector.tensor_scalar_add(dst, src, er_b3_val)
                    nc.sync.dma_start(out=vals_d[0, s:s + 1, :], in_=t_vs[:])

            # ================= phase D: assemble symmetric adjacency =================
            with tc.tile_pool(name="pT2", bufs=2, space="PSUM") as pT2:
                for i in range(NN - 1):
                    w = NN - 1 - i
                    nc.sync.dma_start(
                        out=t_adjU[i:i + 1, :, i + 1:NN],
                        in_=vals_d[:, :, int(_BASE[i]):int(_BASE[i]) + w])
                t_adj = ap_.tile([NN, BL, NN], F32, tag="adjF", name="t_adj")
                for g in range(2):
                    ps = pT2.tile([NN, 512], F32, tag="tp", name="pst")
                    for s8 in range(8):
                        s = g * 8 + s8
                        nc.tensor.transpose(ps[:, s8 * NN:(s8 + 1) * NN],
                                            t_adjU[:, s, :], ident[:])
                    nc.vector.tensor_add(
                        t_adj[:, g * 8:(g + 1) * 8, :].rearrange("p s n -> p (s n)"),
                        t_adjU[:, g * 8:(g + 1) * 8, :].rearrange("p s n -> p (s n)"),
                        ps[:, 0:512])
                nc.sync.dma_start(
                    out=out_adj[:, :].rearrange("s (i j) -> i s j", i=NN),
                    in_=t_adj[:])

    nc.compile()
    return nc


def kernel(**inputs):
    d = {k: np.ascontiguousarray(np.asarray(v, dtype=np.float32)) for k, v in inputs.items()}

    incm = np.zeros((NN + 1, E), np.float32)
    incm[_II, np.arange(E)] = 1.0
    incm[_JJ, np.arange(E)] = 1.0
    incm[NN, :] = 1.0

    shared = {
        "nr_w1": d["nr_w1"], "nr_w2": d["nr_w2"], "nr_w3": d["nr_w3"],
        "nr_b1": np.ascontiguousarray(d["nr_b1"].reshape(4, 128).T),
        "nr_b2": np.ascontiguousarray(d["nr_b2"].reshape(8, 128).T),
        "nr_b3": d["nr_b3"].reshape(1, 1088),
        "ne_w1": d["ne_w1"], "ne_w2": d["ne_w2"],
        "ne_b1": d["ne_b1"].reshape(128, 1), "ne_b2": d["ne_b2"].reshape(128, 1),
        "pl_w1": d["pl_w1"], "pl_w2": d["pl_w2"],
        "pl_b1": np.ascontiguousarray(d["pl_b1"].reshape(2, 128).T),
        "pl_b2": d["pl_b2"].reshape(128, 1),
        "w1a": np.ascontiguousarray(0.5 * d["er_w1"][:128, :]),
        "w1b": np.ascontiguousarray(d["er_w1"][128:, :]),
        "er_b1": d["er_b1"].reshape(1, 256),
        "er_w2": d["er_w2"],
        "er_b2": np.ascontiguousarray(d["er_b2"].reshape(2, 128).T),
        "er_w3": d["er_w3"],
        "incm": incm,
        "ones16": np.ones((1, BL), np.float32),
    }
    in_maps = []
    for c in range(N_CORES):
        m = dict(shared)
        m["dataT"] = np.ascontiguousarray(d["data"][c * BL:(c + 1) * BL, :].T)
        in_maps.append(m)

    nc = _build_program(float(d["er_b3"].reshape(-1)[0]))
    res = run_bass_kernel_spmd(nc, in_maps, list(range(N_CORES)))

    nodes = np.concatenate(
        [res.results[c]["nodes"].reshape(BL, NN, NF + 1) for c in range(N_CORES)], axis=0)
    adj = np.concatenate(
        [res.results[c]["adj"].reshape(BL, NN, NN) for c in range(N_CORES)], axis=0)
    return nodes, adj
